# revision 47
# baseline (speedup 1.0000x reference)
"""Trainium2 Bass kernel for MoE-LoRA fused QKV projection.

Computes, for x[32,512,1024], weight[3072,1024], per-sample top-2 LoRA
expert pools (rank 16) and expert biases:

    qkv = x @ W.T + bias
    qkv[..., :1024]  += SCALE * sum_k attn[b,k] * (x @ A_q[idx]) @ B_q[idx]
    qkv[..., 2048:]  += SCALE * sum_k attn[b,k] * (x @ A_v[idx]) @ B_v[idx]
    qkv += SCALE * sum_k attn[b,k] * bias_pool[idx]

Strategy: data-parallel over batch on 8 NeuronCores (4 samples/core).
Host side: transpose x / weight, gather + scale the tiny LoRA pools per
sample, fold all bias terms into one per-sample row vector that rides the
LoRA matmul via an appended ones-row (rank-1 matmul trick). Device side:
everything is dense fp32r (TF32-like, full PE rate) matmuls accumulated
in PSUM.
"""

import sys

for _p in ("/opt/trn_rl_repo",):
    if _p not in sys.path:
        sys.path.append(_p)

from contextlib import ExitStack

import numpy as np

import concourse.bass as bass
import concourse.tile as tile
from concourse import bacc, mybir
from concourse.bass_utils import run_bass_kernel_spmd

DIM = 1024
RANK = 16
TOPK = 2
BSZ = 32
SEQ = 512
SCALE = 1.0
NCORES = 8
SPC = BSZ // NCORES  # samples per core
NT = SEQ // 128  # token tiles per sample
KT = DIM // 128  # contraction tiles
NC3 = 3 * DIM
NCH = NC3 // 512  # output column chunks

F32 = mybir.dt.float32
F32R = mybir.dt.float32r

_NC_CACHE = {}


def _build_nc():
    nc = bacc.Bacc("TRN2", target_bir_lowering=False, debug=False)
    # all large inputs are pre-packed host-side into exact SBUF layouts so
    # every DMA is 2D-contiguous with multi-KB descriptors
    xT = nc.dram_tensor("xT", [SPC, 128, KT * SEQ], F32R, kind="ExternalInput").ap()
    wT = nc.dram_tensor("wT", [128, 3 * KT * 1024], F32R, kind="ExternalInput").ap()
    acat = nc.dram_tensor(
        "acat", [SPC, 128, KT * 4 * RANK], F32R, kind="ExternalInput"
    ).ap()
    bq = nc.dram_tensor("bq", [SPC, 33, DIM], F32R, kind="ExternalInput").ap()
    bv = nc.dram_tensor("bv", [SPC, 33, DIM], F32R, kind="ExternalInput").ap()
    bk = nc.dram_tensor("bk", [SPC, 128, DIM], F32, kind="ExternalInput").ap()
    ones = nc.dram_tensor("ones", [2, SEQ], F32R, kind="ExternalInput").ap()
    out = nc.dram_tensor("out", [SPC, SEQ, NC3], F32, kind="ExternalOutput").ap()

    with tile.TileContext(nc) as tc, ExitStack() as ctx:
        wpool = ctx.enter_context(tc.tile_pool(name="w", bufs=1))
        xpool = ctx.enter_context(tc.tile_pool(name="x", bufs=2))
        apool = ctx.enter_context(tc.tile_pool(name="a", bufs=2))
        bpool = ctx.enter_context(tc.tile_pool(name="b", bufs=2))
        tpool = ctx.enter_context(tc.tile_pool(name="t", bufs=2))
        opool = ctx.enter_context(tc.tile_pool(name="o", bufs=4))
        pst = ctx.enter_context(tc.tile_pool(name="pst", bufs=1, space="PSUM"))
        pso = ctx.enter_context(tc.tile_pool(name="pso", bufs=6, space="PSUM"))

        def load_sample(s):
            # A_cat first (tiny, unblocks the first t-matmul earliest)
            a_t = apool.tile([128, KT * 4 * RANK], F32R, tag="a")
            nc.sync.dma_start(a_t[:], acat[s])
            # x in two 1 MB halves so the first t-matmuls start early
            x_t = xpool.tile([128, KT * SEQ], F32R, tag="x")
            hx = KT * SEQ // 2
            for i in range(2):
                nc.sync.dma_start(
                    x_t[:, i * hx : (i + 1) * hx], xT[s, :, i * hx : (i + 1) * hx]
                )
            bq_sb = bpool.tile([33, DIM], F32R, tag="bq")
            nc.scalar.dma_start(bq_sb[:], bq[s])
            bv_sb = bpool.tile([33, DIM], F32R, tag="bv")
            nc.scalar.dma_start(bv_sb[:], bv[s])
            # k-part bias pre-broadcast across all 128 partitions: added
            # during PSUM evacuation (saves a PE matmul per k-chunk group)
            bk_sb = bpool.tile([128, DIM], F32, tag="bk")
            nc.scalar.dma_start(bk_sb[:], bk[s])
            return x_t, a_t, bq_sb, bv_sb, bk_sb

        loaded = {0: load_sample(0)}

        # Resident base weight: three 4 MB chunk-pair DMAs into one tile;
        # free layout (cp, k, col): w for (k, cp) at cp*8*1024 + k*1024
        w_all = wpool.tile([128, 3 * KT * 1024], F32R, tag="wall")
        wq = KT * 1024
        for h2 in range(2 * (NCH // 2)):
            nc.sync.dma_start(
                w_all[:, h2 * wq // 2 : (h2 + 1) * wq // 2],
                wT[:, h2 * wq // 2 : (h2 + 1) * wq // 2],
            )

        def wtile(k, cp):
            off = cp * wq + k * 1024
            return w_all[:, off : off + 1024]

        for s in range(SPC):
            if s not in loaded:
                loaded[s] = load_sample(s)
            x_t, a_t, bq_sb, bv_sb, bk_sb = loaded.pop(s)

            # tq/tv rows 0-31 = t, row 32 = ones (bias rides rank-1 row)
            tq_sb = tpool.tile([33, SEQ], F32R, tag="tq")
            tv_sb = tpool.tile([33, SEQ], F32R, tag="tv")
            nc.scalar.dma_start(tq_sb[32:33, :], ones[0:1, :])
            nc.scalar.dma_start(tv_sb[32:33, :], ones[0:1, :])

            o_tiles = {}

            def ensure_o(cp, nt):
                if (cp, nt) not in o_tiles:
                    o_tiles[(cp, nt)] = opool.tile([128, 1024], F32, tag="o", name="o")
                return o_tiles[(cp, nt)]

            # Every PSUM accumulation group is modelled as a list of matmul
            # thunks + an evacuation thunk; groups are chained with the
            # next group's first two matmuls staggered into this group's
            # tail so fresh-bank issue penalties overlap streaming.
            def make_t_group(off, dst):
                pt = pst.tile([32, SEQ], F32, tag=f"pt{off}", name="pt")

                def mk(k):
                    def f():
                        nc.tensor.matmul(
                            pt[:],
                            a_t[:, k * 64 + off : k * 64 + off + 32],
                            x_t[:, bass.ts(k, SEQ)],
                            start=(k == 0),
                            stop=(k == KT - 1),
                            skip_group_check=True,
                        )

                    return f

                def ev():
                    nc.vector.tensor_copy(dst[0:32, :], pt[:])

                return {"mms": [mk(k) for k in range(KT)], "evac": ev, "done": 0}

            def make_chunk_group(cp, nt, half):
                c = cp * 2 + half
                tok = bass.ts(nt, 128)
                po = pso.tile([128, 512], F32, tag="po", name="po")
                mms = []
                if c < 2:
                    lhsT, rhs = tq_sb[:, tok], bq_sb[:, bass.ts(c, 512)]
                elif c >= 4:
                    lhsT, rhs = tv_sb[:, tok], bv_sb[:, bass.ts(c - 4, 512)]
                else:
                    lhsT = rhs = None
                if lhsT is not None:

                    def lora():
                        nc.tensor.matmul(
                            po[:], lhsT, rhs,
                            start=True, stop=False, skip_group_check=True,
                        )

                    mms.append(lora)

                def mk(k):
                    def f():
                        nc.tensor.matmul(
                            po[:],
                            x_t[:, k * SEQ + nt * 128 : k * SEQ + (nt + 1) * 128],
                            wtile(k, cp)[:, bass.ts(half, 512)],
                            start=(lhsT is None and k == 0),
                            stop=(k == KT - 1),
                            skip_group_check=True,
                        )

                    return f

                mms.extend(mk(k) for k in range(KT))

                def ev():
                    o_sb = ensure_o(cp, nt)
                    if 2 <= c < 4:
                        # k-part: bias added during evacuation (DVE)
                        nc.vector.tensor_add(
                            o_sb[:, bass.ts(half, 512)], po[:],
                            bk_sb[:, bass.ts(c - 2, 512)],
                        )
                    else:
                        # split evacuation across DVE and ACT to shorten
                        # the PSUM-read window
                        nc.vector.tensor_copy(
                            o_sb[:, half * 512 : half * 512 + 256], po[:, 0:256]
                        )
                        nc.scalar.copy(
                            o_sb[:, half * 512 + 256 : half * 512 + 512],
                            po[:, 256:512],
                        )
                    if half == 1:
                        nc.scalar.dma_start(
                            out[s, bass.ts(nt, 128), bass.ts(cp, 1024)], o_sb[:]
                        )
                        del o_tiles[(cp, nt)]

                return {"mms": mms, "evac": ev, "done": 0}

            sgroups = [make_t_group(0, tq_sb), make_t_group(32, tv_sb)]
            for cp in range(NCH // 2):
                for nt in range(NT):
                    for half in range(2):
                        sgroups.append(make_chunk_group(cp, nt, half))

            for gi, g in enumerate(sgroups):
                nxt = sgroups[gi + 1] if gi + 1 < len(sgroups) else None
                n = len(g["mms"])
                for j in range(g["done"], n):
                    rem = n - j
                    if nxt is not None and rem == 4:
                        nxt["mms"][0]()
                        nxt["done"] = 1
                    elif nxt is not None and rem == 2 and nxt["done"] == 1:
                        nxt["mms"][1]()
                        nxt["done"] = 2
                    g["mms"][j]()
                g["evac"]()

    nc.compile()
    return nc


def _get_nc():
    if "nc" not in _NC_CACHE:
        _NC_CACHE["nc"] = _build_nc()
    return _NC_CACHE["nc"]


def kernel(**inputs):
    x = np.asarray(inputs["x"], dtype=np.float32)
    weight = np.asarray(inputs["weight"], dtype=np.float32)
    bias = np.asarray(inputs["bias"], dtype=np.float32)
    A_q = np.asarray(inputs["A_q_pool"], dtype=np.float32)
    B_q = np.asarray(inputs["B_q_pool"], dtype=np.float32)
    A_v = np.asarray(inputs["A_v_pool"], dtype=np.float32)
    B_v = np.asarray(inputs["B_v_pool"], dtype=np.float32)
    bias_pool = np.asarray(inputs["bias_pool"], dtype=np.float32)
    attn = np.asarray(inputs["attn"], dtype=np.float32)
    idx = np.asarray(inputs["idx"]).astype(np.int64)

    # -- host-side prep: pack the big operands into exact SBUF layouts --
    # x[b, n, k*128+p] -> xT[b, p, k*SEQ + n]
    xT = np.ascontiguousarray(
        x.reshape(BSZ, SEQ, KT, 128).transpose(0, 3, 2, 1).reshape(BSZ, 128, KT * SEQ)
    )
    # weight.T[k*128+p, cp*1024+col] -> wT[p, cp*8192 + k*1024 + col]
    wT = np.ascontiguousarray(
        weight.T.reshape(KT, 128, 3, 1024)
        .transpose(1, 2, 0, 3)
        .reshape(128, 3 * KT * 1024)
    )

    i0, i1 = idx[:, 0], idx[:, 1]
    # [B, DIM, 64]: columns 0-15 q/k0, 16-31 q/k1, 32-47 v/k0, 48-63 v/k1
    acat = np.concatenate([A_q[i0], A_q[i1], A_v[i0], A_v[i1]], axis=2)
    # acat[b, k*128+p, r] -> [b, p, k*64 + r]
    acat = np.ascontiguousarray(
        acat.reshape(BSZ, KT, 128, 4 * RANK)
        .transpose(0, 2, 1, 3)
        .reshape(BSZ, 128, KT * 4 * RANK)
    )

    wgt = (SCALE * attn)[:, :, None, None]  # [B, K, 1, 1]
    bq_s = (B_q[idx] * wgt).reshape(BSZ, TOPK * RANK, DIM)
    bv_s = (B_v[idx] * wgt).reshape(BSZ, TOPK * RANK, DIM)
    bias_total = bias[None, :] + SCALE * np.einsum(
        "bko,bk->bo", bias_pool[idx], attn
    )  # [B, 3*DIM]
    bq_ext = np.ascontiguousarray(
        np.concatenate([bq_s, bias_total[:, None, :DIM]], axis=1)
    )  # [B, 33, DIM]
    bv_ext = np.ascontiguousarray(
        np.concatenate([bv_s, bias_total[:, None, 2 * DIM :]], axis=1)
    )  # [B, 33, DIM]
    # k-part bias replicated across the 128 token partitions
    bk_row = np.ascontiguousarray(
        np.broadcast_to(bias_total[:, None, DIM : 2 * DIM], (BSZ, 128, DIM))
    )  # [B, 128, DIM]
    ones = np.ones((2, SEQ), dtype=np.float32)

    nc = _get_nc()
    in_maps = []
    for c in range(NCORES):
        sl = slice(c * SPC, (c + 1) * SPC)
        in_maps.append(
            {
                "xT": xT[sl],
                "wT": wT,
                "acat": acat[sl],
                "bq": bq_ext[sl],
                "bv": bv_ext[sl],
                "bk": bk_row[sl],
                "ones": ones,
            }
        )
    global _LAST_IN_MAPS
    _LAST_IN_MAPS = in_maps
    res = run_bass_kernel_spmd(nc, in_maps, list(range(NCORES)))
    out = np.concatenate(
        [np.asarray(res.results[i]["out"]) for i in range(NCORES)], axis=0
    )
    return out.astype(np.float32, copy=False)


_LAST_IN_MAPS = None


# revision 48
# speedup vs baseline: 1.0426x; 1.0426x over previous
"""Trainium2 Bass kernel for MoE-LoRA fused QKV projection.

Computes, for x[32,512,1024], weight[3072,1024], per-sample top-2 LoRA
expert pools (rank 16) and expert biases:

    qkv = x @ W.T + bias
    qkv[..., :1024]  += SCALE * sum_k attn[b,k] * (x @ A_q[idx]) @ B_q[idx]
    qkv[..., 2048:]  += SCALE * sum_k attn[b,k] * (x @ A_v[idx]) @ B_v[idx]
    qkv += SCALE * sum_k attn[b,k] * bias_pool[idx]

Strategy: data-parallel over batch on 8 NeuronCores (4 samples/core).
Host side: transpose x / weight, gather + scale the tiny LoRA pools per
sample, fold all bias terms into one per-sample row vector that rides the
LoRA matmul via an appended ones-row (rank-1 matmul trick). Device side:
everything is dense fp32r (TF32-like, full PE rate) matmuls accumulated
in PSUM.
"""

import sys

for _p in ("/opt/trn_rl_repo",):
    if _p not in sys.path:
        sys.path.append(_p)

from contextlib import ExitStack

import numpy as np

import concourse.bass as bass
import concourse.tile as tile
from concourse import bacc, mybir
from concourse.bass_utils import run_bass_kernel_spmd

DIM = 1024
RANK = 16
TOPK = 2
BSZ = 32
SEQ = 512
SCALE = 1.0
NCORES = 8
SPC = BSZ // NCORES  # samples per core
NT = SEQ // 128  # token tiles per sample
KT = DIM // 128  # contraction tiles
NC3 = 3 * DIM
NCH = NC3 // 512  # output column chunks

F32 = mybir.dt.float32
F32R = mybir.dt.float32r

_NC_CACHE = {}


def _build_nc():
    nc = bacc.Bacc("TRN2", target_bir_lowering=False, debug=False)
    # all large inputs are pre-packed host-side into exact SBUF layouts so
    # every DMA is 2D-contiguous with multi-KB descriptors
    xT = nc.dram_tensor("xT", [SPC, 128, KT * SEQ], F32R, kind="ExternalInput").ap()
    wT = nc.dram_tensor("wT", [128, 3 * KT * 1024], F32R, kind="ExternalInput").ap()
    acat = nc.dram_tensor(
        "acat", [SPC, 128, KT * 4 * RANK], F32R, kind="ExternalInput"
    ).ap()
    bq = nc.dram_tensor("bq", [SPC, 33, DIM], F32R, kind="ExternalInput").ap()
    bv = nc.dram_tensor("bv", [SPC, 33, DIM], F32R, kind="ExternalInput").ap()
    bk = nc.dram_tensor("bk", [SPC, 128, DIM], F32, kind="ExternalInput").ap()
    ones = nc.dram_tensor("ones", [2, SEQ], F32R, kind="ExternalInput").ap()
    out = nc.dram_tensor("out", [SPC, SEQ, NC3], F32, kind="ExternalOutput").ap()

    with tile.TileContext(nc) as tc, ExitStack() as ctx:
        wpool = ctx.enter_context(tc.tile_pool(name="w", bufs=1))
        xpool = ctx.enter_context(tc.tile_pool(name="x", bufs=2))
        apool = ctx.enter_context(tc.tile_pool(name="a", bufs=2))
        bpool = ctx.enter_context(tc.tile_pool(name="b", bufs=2))
        tpool = ctx.enter_context(tc.tile_pool(name="t", bufs=2))
        opool = ctx.enter_context(tc.tile_pool(name="o", bufs=4))
        pst = ctx.enter_context(tc.tile_pool(name="pst", bufs=1, space="PSUM"))
        pso = ctx.enter_context(tc.tile_pool(name="pso", bufs=6, space="PSUM"))

        def load_sample(s):
            # A_cat first (tiny, unblocks the first t-matmul earliest)
            a_t = apool.tile([128, KT * 4 * RANK], F32R, tag="a")
            nc.sync.dma_start(a_t[:], acat[s])
            # x in two 1 MB halves so the first t-matmuls start early
            x_t = xpool.tile([128, KT * SEQ], F32R, tag="x")
            hx = KT * SEQ // 2
            for i in range(2):
                nc.sync.dma_start(
                    x_t[:, i * hx : (i + 1) * hx], xT[s, :, i * hx : (i + 1) * hx]
                )
            bq_sb = bpool.tile([33, DIM], F32R, tag="bq")
            nc.scalar.dma_start(bq_sb[:], bq[s])
            bv_sb = bpool.tile([33, DIM], F32R, tag="bv")
            nc.scalar.dma_start(bv_sb[:], bv[s])
            # k-part bias pre-broadcast across all 128 partitions: added
            # during PSUM evacuation (saves a PE matmul per k-chunk group)
            bk_sb = bpool.tile([128, DIM], F32, tag="bk")
            nc.scalar.dma_start(bk_sb[:], bk[s])
            return x_t, a_t, bq_sb, bv_sb, bk_sb

        loaded = {0: load_sample(0)}

        # Resident base weight: three 4 MB chunk-pair DMAs into one tile;
        # free layout (cp, k, col): w for (k, cp) at cp*8*1024 + k*1024
        w_all = wpool.tile([128, 3 * KT * 1024], F32R, tag="wall")
        wq = KT * 1024
        for h2 in range(2 * (NCH // 2)):
            nc.sync.dma_start(
                w_all[:, h2 * wq // 2 : (h2 + 1) * wq // 2],
                wT[:, h2 * wq // 2 : (h2 + 1) * wq // 2],
            )

        def wtile(k, cp):
            off = cp * wq + k * 1024
            return w_all[:, off : off + 1024]

        for s in range(SPC):
            if s not in loaded:
                loaded[s] = load_sample(s)
            x_t, a_t, bq_sb, bv_sb, bk_sb = loaded.pop(s)

            # tq/tv rows 0-31 = t, row 32 = ones (bias rides rank-1 row)
            tq_sb = tpool.tile([33, SEQ], F32R, tag="tq")
            tv_sb = tpool.tile([33, SEQ], F32R, tag="tv")
            nc.scalar.dma_start(tq_sb[32:33, :], ones[0:1, :])
            nc.scalar.dma_start(tv_sb[32:33, :], ones[0:1, :])

            o_tiles = {}

            def ensure_o(cp, nt):
                if (cp, nt) not in o_tiles:
                    o_tiles[(cp, nt)] = opool.tile([128, 1024], F32, tag="o", name="o")
                return o_tiles[(cp, nt)]

            # Every PSUM accumulation group is modelled as a list of matmul
            # thunks + an evacuation thunk; groups are chained with the
            # next group's first two matmuls staggered into this group's
            # tail so fresh-bank issue penalties overlap streaming.
            def make_t_group(off, dst):
                pt = pst.tile([32, SEQ], F32, tag=f"pt{off}", name="pt")

                def mk(k):
                    def f():
                        nc.tensor.matmul(
                            pt[:],
                            a_t[:, k * 64 + off : k * 64 + off + 32],
                            x_t[:, bass.ts(k, SEQ)],
                            start=(k == 0),
                            stop=(k == KT - 1),
                            skip_group_check=True,
                        )

                    return f

                def ev():
                    nc.vector.tensor_copy(dst[0:32, :], pt[:])

                return {"mms": [mk(k) for k in range(KT)], "evac": ev, "done": 0}

            def make_chunk_group(cp, nt, half):
                c = cp * 2 + half
                tok = bass.ts(nt, 128)
                po = pso.tile([128, 512], F32, tag="po", name="po")
                mms = []
                if c < 2:
                    lhsT, rhs = tq_sb[:, tok], bq_sb[:, bass.ts(c, 512)]
                elif c >= 4:
                    lhsT, rhs = tv_sb[:, tok], bv_sb[:, bass.ts(c - 4, 512)]
                else:
                    lhsT = rhs = None
                if lhsT is not None:

                    def lora():
                        nc.tensor.matmul(
                            po[:], lhsT, rhs,
                            start=True, stop=False, skip_group_check=True,
                        )

                    mms.append(lora)

                def mk(k):
                    def f():
                        nc.tensor.matmul(
                            po[:],
                            x_t[:, k * SEQ + nt * 128 : k * SEQ + (nt + 1) * 128],
                            wtile(k, cp)[:, bass.ts(half, 512)],
                            start=(lhsT is None and k == 0),
                            stop=(k == KT - 1),
                            skip_group_check=True,
                        )

                    return f

                mms.extend(mk(k) for k in range(KT))

                def ev():
                    o_sb = ensure_o(cp, nt)
                    if 2 <= c < 4:
                        # k-part: bias added during evacuation (DVE)
                        nc.vector.tensor_add(
                            o_sb[:, bass.ts(half, 512)], po[:],
                            bk_sb[:, bass.ts(c - 2, 512)],
                        )
                    else:
                        # split evacuation across DVE and ACT to shorten
                        # the PSUM-read window
                        nc.vector.tensor_copy(
                            o_sb[:, half * 512 : half * 512 + 256], po[:, 0:256]
                        )
                        nc.scalar.copy(
                            o_sb[:, half * 512 + 256 : half * 512 + 512],
                            po[:, 256:512],
                        )
                    if half == 1:
                        nc.scalar.dma_start(
                            out[s, bass.ts(nt, 128), bass.ts(cp, 1024)], o_sb[:]
                        )
                        del o_tiles[(cp, nt)]

                return {"mms": mms, "evac": ev, "done": 0}

            sgroups = [make_t_group(0, tq_sb), make_t_group(32, tv_sb)]
            for cp in range(NCH // 2):
                for nt in range(NT):
                    for half in range(2):
                        sgroups.append(make_chunk_group(cp, nt, half))

            for gi, g in enumerate(sgroups):
                nxt = sgroups[gi + 1] if gi + 1 < len(sgroups) else None
                n = len(g["mms"])
                for j in range(g["done"], n):
                    rem = n - j
                    if nxt is not None and rem == 3:
                        nxt["mms"][0]()
                        nxt["done"] = 1
                    elif nxt is not None and rem == 1 and nxt["done"] == 1:
                        nxt["mms"][1]()
                        nxt["done"] = 2
                    g["mms"][j]()
                g["evac"]()

    nc.compile()
    return nc


def _get_nc():
    if "nc" not in _NC_CACHE:
        _NC_CACHE["nc"] = _build_nc()
    return _NC_CACHE["nc"]


def kernel(**inputs):
    x = np.asarray(inputs["x"], dtype=np.float32)
    weight = np.asarray(inputs["weight"], dtype=np.float32)
    bias = np.asarray(inputs["bias"], dtype=np.float32)
    A_q = np.asarray(inputs["A_q_pool"], dtype=np.float32)
    B_q = np.asarray(inputs["B_q_pool"], dtype=np.float32)
    A_v = np.asarray(inputs["A_v_pool"], dtype=np.float32)
    B_v = np.asarray(inputs["B_v_pool"], dtype=np.float32)
    bias_pool = np.asarray(inputs["bias_pool"], dtype=np.float32)
    attn = np.asarray(inputs["attn"], dtype=np.float32)
    idx = np.asarray(inputs["idx"]).astype(np.int64)

    # -- host-side prep: pack the big operands into exact SBUF layouts --
    # x[b, n, k*128+p] -> xT[b, p, k*SEQ + n]
    xT = np.ascontiguousarray(
        x.reshape(BSZ, SEQ, KT, 128).transpose(0, 3, 2, 1).reshape(BSZ, 128, KT * SEQ)
    )
    # weight.T[k*128+p, cp*1024+col] -> wT[p, cp*8192 + k*1024 + col]
    wT = np.ascontiguousarray(
        weight.T.reshape(KT, 128, 3, 1024)
        .transpose(1, 2, 0, 3)
        .reshape(128, 3 * KT * 1024)
    )

    i0, i1 = idx[:, 0], idx[:, 1]
    # [B, DIM, 64]: columns 0-15 q/k0, 16-31 q/k1, 32-47 v/k0, 48-63 v/k1
    acat = np.concatenate([A_q[i0], A_q[i1], A_v[i0], A_v[i1]], axis=2)
    # acat[b, k*128+p, r] -> [b, p, k*64 + r]
    acat = np.ascontiguousarray(
        acat.reshape(BSZ, KT, 128, 4 * RANK)
        .transpose(0, 2, 1, 3)
        .reshape(BSZ, 128, KT * 4 * RANK)
    )

    wgt = (SCALE * attn)[:, :, None, None]  # [B, K, 1, 1]
    bq_s = (B_q[idx] * wgt).reshape(BSZ, TOPK * RANK, DIM)
    bv_s = (B_v[idx] * wgt).reshape(BSZ, TOPK * RANK, DIM)
    bias_total = bias[None, :] + SCALE * np.einsum(
        "bko,bk->bo", bias_pool[idx], attn
    )  # [B, 3*DIM]
    bq_ext = np.ascontiguousarray(
        np.concatenate([bq_s, bias_total[:, None, :DIM]], axis=1)
    )  # [B, 33, DIM]
    bv_ext = np.ascontiguousarray(
        np.concatenate([bv_s, bias_total[:, None, 2 * DIM :]], axis=1)
    )  # [B, 33, DIM]
    # k-part bias replicated across the 128 token partitions
    bk_row = np.ascontiguousarray(
        np.broadcast_to(bias_total[:, None, DIM : 2 * DIM], (BSZ, 128, DIM))
    )  # [B, 128, DIM]
    ones = np.ones((2, SEQ), dtype=np.float32)

    nc = _get_nc()
    in_maps = []
    for c in range(NCORES):
        sl = slice(c * SPC, (c + 1) * SPC)
        in_maps.append(
            {
                "xT": xT[sl],
                "wT": wT,
                "acat": acat[sl],
                "bq": bq_ext[sl],
                "bv": bv_ext[sl],
                "bk": bk_row[sl],
                "ones": ones,
            }
        )
    global _LAST_IN_MAPS
    _LAST_IN_MAPS = in_maps
    res = run_bass_kernel_spmd(nc, in_maps, list(range(NCORES)))
    out = np.concatenate(
        [np.asarray(res.results[i]["out"]) for i in range(NCORES)], axis=0
    )
    return out.astype(np.float32, copy=False)


_LAST_IN_MAPS = None
